# revision 14
# baseline (speedup 1.0000x reference)
"""GCN layer kernel for Trainium2, 8-core row-parallel.

Computes out = (adj * mask + I) @ (x @ W^T) for N=8192, C_in=C_out=128.

Sharding: adj/mask row-blocks of 1024 across 8 cores; x, W replicated.
Per core pipeline (heavy matmul work in fp32r = FP22-truncated fp32,
~1e-4 relative error, single-pass on the PE):
  - h = x @ W^T on-chip; x tiles transposed via regular identity matmuls
  - adj/mask row-block streamed in 1MB chunks, multiplied on DVE,
    product tiles transposed via regular identity matmuls on the PE
    (counts as PE-busy, keeps the HAM clock gate warm), PSUM->SBUF copies
    on ACT, then fp32r matmuls (stationary = h k-tile, moving = A^T
    [128k, 512m]) accumulate out^T in PSUM
  - self-loop +h fused into the finalize add after back-transposing out^T
  - loads stream on the SP DMA queue; x/out use the ACT queue so finalize
    writes never block the load FIFO
"""

import numpy as np
from contextlib import ExitStack

from concourse import bass, bacc, tile, mybir
from concourse import masks
from concourse.bass_utils import run_bass_kernel_spmd

N = 8192
C = 128
NCORES = 8
R = N // NCORES          # 1024 rows per core
M_BLK = 512              # psum accumulation block (free dim of main matmul)
NBLK = R // M_BLK        # 2 m-blocks per core
S = M_BLK // 128         # 4 slabs of 128 rows per m-block
KQ = 1024                # k-chunk width per DMA iteration
NQ = N // KQ             # 8 k-chunks
KT = KQ // 128           # 8 k-tiles per chunk
NKT = N // 128           # 64 k-tiles total

F32 = mybir.dt.float32
F32R = mybir.dt.float32r


def build_program():
    nc = bacc.Bacc("TRN2", target_bir_lowering=False, debug=False, num_devices=NCORES)

    adj_d = nc.dram_tensor("adj", [R, N], F32, kind="ExternalInput").ap()
    mask_d = nc.dram_tensor("mask", [R, N], F32, kind="ExternalInput").ap()
    x_d = nc.dram_tensor("x", [N, C], F32, kind="ExternalInput").ap()
    xo_d = nc.dram_tensor("x_own", [R, C], F32, kind="ExternalInput").ap()
    w_d = nc.dram_tensor("w", [C, C], F32, kind="ExternalInput").ap()
    out_d = nc.dram_tensor("out", [R, C], F32, kind="ExternalOutput").ap()

    with tile.TileContext(nc) as tc, ExitStack() as ctx:
        const_pool = ctx.enter_context(tc.tile_pool(name="const", bufs=1))
        xr_pool = ctx.enter_context(tc.tile_pool(name="xr", bufs=2))
        xt_pool = ctx.enter_context(tc.tile_pool(name="xt", bufs=3))
        h_pool = ctx.enter_context(tc.tile_pool(name="h", bufs=1))
        adj_pool = ctx.enter_context(tc.tile_pool(name="adj", bufs=5))
        mask_pool = ctx.enter_context(tc.tile_pool(name="mask", bufs=5))
        prod_pool = ctx.enter_context(tc.tile_pool(name="prod", bufs=5))
        at_pool = ctx.enter_context(tc.tile_pool(name="at", bufs=6))
        fin_pool = ctx.enter_context(tc.tile_pool(name="fin", bufs=2))
        psum_acc = ctx.enter_context(tc.tile_pool(name="pacc", bufs=2, space="PSUM"))
        psum_tr = ctx.enter_context(tc.tile_pool(name="ptr", bufs=3, space="PSUM"))
        psum_misc = ctx.enter_context(tc.tile_pool(name="pmisc", bufs=2, space="PSUM"))
        psum_fin = ctx.enter_context(tc.tile_pool(name="pfin", bufs=1, space="PSUM"))

        ident = const_pool.tile([128, 128], F32)
        masks.make_identity(nc, ident[:])
        identr = const_pool.tile([128, 128], F32R)
        nc.vector.tensor_copy(identr[:], ident[:])

        # ---- Phase 0: h = x @ W^T ----
        w_sb = const_pool.tile([128, C], F32)
        nc.scalar.dma_start(out=w_sb[:], in_=w_d[:, :])
        psum_wt = psum_misc.tile([128, 128], F32, tag="pm")
        nc.tensor.transpose(psum_wt[:], w_sb[:], ident[:])
        wtr_sb = const_pool.tile([128, C], F32R)
        nc.vector.tensor_copy(wtr_sb[:], psum_wt[:])

        h_sb = h_pool.tile([128, NKT, C], F32R)
        XCH = 16  # x DMA chunks so h-compute overlaps the load
        NTX = NKT // XCH  # 4 tiles = 512 rows per chunk
        for xc in range(XCH):
            x_raw = xr_pool.tile([128, NTX, C], F32, tag="xraw")
            nc.scalar.dma_start(
                out=x_raw[:],
                in_=x_d[xc * NTX * 128 : (xc + 1) * NTX * 128, :].rearrange(
                    "(t p) c -> p t c", p=128
                ),
            )
            x_rnd = xr_pool.tile([128, NTX, C], F32R, tag="xrnd")
            nc.vector.tensor_copy(x_rnd[:], x_raw[:])  # fp32r rounding pass
            # x^T for the whole 512-row chunk via 4 fast transposes
            psum_xtw = psum_misc.tile([128, NTX * C], F32R, tag="pm")
            for tt in range(NTX):
                nc.tensor.transpose(
                    psum_xtw[:, tt * C : (tt + 1) * C], x_rnd[:, tt, :], identr[:]
                )
            xt_wide = xt_pool.tile([128, NTX * C], F32R, tag="xtw")
            nc.scalar.copy(xt_wide[:], psum_xtw[:])
            # h^T chunk in one wide matmul: [c, 512] = W^T.T @ x^T
            psum_ht = psum_misc.tile([128, NTX * C], F32, tag="pm")
            nc.tensor.matmul(psum_ht[:], wtr_sb[:], xt_wide[:], start=True, stop=True)
            hT_sb = xt_pool.tile([128, NTX * C], F32R, tag="htw")
            nc.vector.tensor_copy(hT_sb[:], psum_ht[:])
            # back to natural h tiles
            for tt in range(NTX):
                t = xc * NTX + tt
                psum_hn = psum_misc.tile([128, 128], F32R, tag="pm")
                nc.tensor.transpose(
                    psum_hn[:], hT_sb[:, tt * C : (tt + 1) * C], identr[:]
                )
                nc.vector.tensor_copy(h_sb[:, t, :], psum_hn[:])

        # h rows owned by this core (for the +I self-loop), exact fp32 path
        xo_sb = xr_pool.tile([128, R // 128, C], F32, tag="xo")
        nc.scalar.dma_start(
            out=xo_sb[:], in_=xo_d.rearrange("(t p) c -> p t c", p=128)
        )
        ho_sb = h_pool.tile([128, R // 128, C], F32)
        for t in range(R // 128):
            psum_xt = psum_misc.tile([128, 128], F32, tag="pm")
            nc.tensor.transpose(psum_xt[:], xo_sb[:, t, :], ident[:])
            xt_f = xt_pool.tile([128, 128], F32, tag="xtf")
            nc.vector.tensor_copy(xt_f[:], psum_xt[:])
            psum_h = psum_misc.tile([128, 128], F32, tag="pm")
            nc.tensor.matmul(
                psum_h[:], xt_f[:], wtr_sb[:].bitcast(F32), start=True, stop=True
            )
            nc.vector.tensor_copy(ho_sb[:, t, :], psum_h[:])

        # ---- Phase 1: main loop over (m-block, k-chunk) ----
        for blk in range(NBLK):
            pacc = psum_acc.tile([128, M_BLK], F32)
            for q in range(NQ):
                # load/multiply in half-chunks of 2 slabs: finer buffer
                # release keeps the DMA queue streaming without stalls
                halves = []
                for hb in range(2):
                    r0 = blk * M_BLK + hb * 256
                    adj_t = adj_pool.tile([128, 2, KQ], F32)
                    nc.sync.dma_start(
                        out=adj_t[:],
                        in_=adj_d[r0 : r0 + 256, q * KQ : (q + 1) * KQ].rearrange(
                            "(s p) k -> p s k", p=128
                        ),
                    )
                    mask_t = mask_pool.tile([128, 2, KQ], F32)
                    nc.sync.dma_start(
                        out=mask_t[:],
                        in_=mask_d[r0 : r0 + 256, q * KQ : (q + 1) * KQ].rearrange(
                            "(s p) k -> p s k", p=128
                        ),
                    )
                    prod_t = prod_pool.tile([128, 2, KQ], F32R)
                    nc.vector.tensor_mul(prod_t[:], adj_t[:], mask_t[:])
                    halves.append(prod_t)

                for kt in range(KT):
                    kg = q * KT + kt  # global k-tile index 0..63
                    psum_at = psum_tr.tile([128, M_BLK], F32R)
                    for s in range(S):
                        nc.tensor.transpose(
                            psum_at[:, s * 128 : (s + 1) * 128],
                            halves[s // 2][:, s % 2, kt * 128 : (kt + 1) * 128],
                            identr[:],
                        )
                    at_sb = at_pool.tile([128, M_BLK], F32R)
                    nc.scalar.copy(at_sb[:], psum_at[:])
                    nc.tensor.matmul(
                        pacc[:],
                        h_sb[:, kg, :],
                        at_sb[:],
                        start=(kg == 0),
                        stop=(kg == NKT - 1),
                    )

            # ---- finalize m-block: back-transpose out^T, add self-loop h ----
            outT_sb = fin_pool.tile([128, M_BLK], F32)
            nc.vector.tensor_copy(outT_sb[:], pacc[:])
            psum_nat = psum_fin.tile([128, M_BLK], F32)
            for s in range(S):
                nc.tensor.transpose(
                    psum_nat[:, s * 128 : (s + 1) * 128],
                    outT_sb[:, s * 128 : (s + 1) * 128],
                    ident[:],
                )
            out_sb = fin_pool.tile([128, S, C], F32)
            nc.vector.tensor_add(
                out_sb[:],
                psum_nat[:].rearrange("p (s c) -> p s c", s=S),
                ho_sb[:, blk * S : (blk + 1) * S, :],
            )
            nc.scalar.dma_start(
                out=out_d[blk * M_BLK : (blk + 1) * M_BLK, :].rearrange(
                    "(s p) c -> p s c", p=128
                ),
                in_=out_sb[:],
            )

    nc.compile()
    return nc


_NC_CACHE = None


def _get_nc():
    global _NC_CACHE
    if _NC_CACHE is None:
        _NC_CACHE = build_program()
    return _NC_CACHE


def kernel(x, adj, mask, W):
    x = np.ascontiguousarray(x, dtype=np.float32)
    adj = np.ascontiguousarray(adj, dtype=np.float32)
    mask = np.ascontiguousarray(mask, dtype=np.float32)
    W = np.ascontiguousarray(W, dtype=np.float32)

    nc = _get_nc()
    in_maps = []
    for i in range(NCORES):
        r0 = i * R
        in_maps.append(
            {
                "adj": adj[r0 : r0 + R],
                "mask": mask[r0 : r0 + R],
                "x": x,
                "x_own": x[r0 : r0 + R],
                "w": W,
            }
        )
    res = run_bass_kernel_spmd(nc, in_maps, list(range(NCORES)))
    return np.concatenate([res.results[i]["out"] for i in range(NCORES)], axis=0)


# revision 15
# speedup vs baseline: 1.1659x; 1.1659x over previous
"""GCN layer kernel for Trainium2, 8-core row-parallel.

Computes out = (adj * mask + I) @ (x @ W^T) for N=8192, C_in=C_out=128.

Sharding: adj/mask row-blocks of 1024 across 8 cores; x, W replicated.
Per core pipeline (heavy matmul work in fp32r = FP22-truncated fp32,
~1e-4 relative error, single-pass on the PE):
  - h = x @ W^T on-chip; x tiles transposed via regular identity matmuls
  - adj/mask row-block streamed in 1MB chunks, multiplied on DVE,
    product tiles transposed via regular identity matmuls on the PE
    (counts as PE-busy, keeps the HAM clock gate warm), PSUM->SBUF copies
    on ACT, then fp32r matmuls (stationary = h k-tile, moving = A^T
    [128k, 512m]) accumulate out^T in PSUM
  - self-loop +h fused into the finalize add after back-transposing out^T
  - loads stream on the SP DMA queue; x/out use the ACT queue so finalize
    writes never block the load FIFO
"""

import numpy as np
from contextlib import ExitStack

from concourse import bass, bacc, tile, mybir
from concourse import masks
from concourse.bass_utils import run_bass_kernel_spmd

N = 8192
C = 128
NCORES = 8
R = N // NCORES          # 1024 rows per core
M_BLK = 512              # psum accumulation block (free dim of main matmul)
NBLK = R // M_BLK        # 2 m-blocks per core
S = M_BLK // 128         # 4 slabs of 128 rows per m-block
KQ = 1024                # k-chunk width per DMA iteration
NQ = N // KQ             # 8 k-chunks
KT = KQ // 128           # 8 k-tiles per chunk
NKT = N // 128           # 64 k-tiles total

F32 = mybir.dt.float32
F32R = mybir.dt.float32r


def build_program():
    nc = bacc.Bacc("TRN2", target_bir_lowering=False, debug=False, num_devices=NCORES)

    adj_d = nc.dram_tensor("adj", [R, N], F32, kind="ExternalInput").ap()
    mask_d = nc.dram_tensor("mask", [R, N], F32, kind="ExternalInput").ap()
    x_d = nc.dram_tensor("x", [N, C], F32, kind="ExternalInput").ap()
    xo_d = nc.dram_tensor("x_own", [R, C], F32, kind="ExternalInput").ap()
    w_d = nc.dram_tensor("w", [C, C], F32, kind="ExternalInput").ap()
    out_d = nc.dram_tensor("out", [R, C], F32, kind="ExternalOutput").ap()

    with tile.TileContext(nc) as tc, ExitStack() as ctx:
        const_pool = ctx.enter_context(tc.tile_pool(name="const", bufs=1))
        xr_pool = ctx.enter_context(tc.tile_pool(name="xr", bufs=2))
        xt_pool = ctx.enter_context(tc.tile_pool(name="xt", bufs=3))
        h_pool = ctx.enter_context(tc.tile_pool(name="h", bufs=1))
        adj_pool = ctx.enter_context(tc.tile_pool(name="adj", bufs=4))
        mask_pool = ctx.enter_context(tc.tile_pool(name="mask", bufs=4))
        prod_pool = ctx.enter_context(tc.tile_pool(name="prod", bufs=4))
        at_pool = ctx.enter_context(tc.tile_pool(name="at", bufs=6))
        fin_pool = ctx.enter_context(tc.tile_pool(name="fin", bufs=2))
        psum_acc = ctx.enter_context(tc.tile_pool(name="pacc", bufs=2, space="PSUM"))
        psum_tr = ctx.enter_context(tc.tile_pool(name="ptr", bufs=3, space="PSUM"))
        psum_misc = ctx.enter_context(tc.tile_pool(name="pmisc", bufs=2, space="PSUM"))
        psum_fin = ctx.enter_context(tc.tile_pool(name="pfin", bufs=1, space="PSUM"))

        ident = const_pool.tile([128, 128], F32)
        masks.make_identity(nc, ident[:])
        identr = const_pool.tile([128, 128], F32R)
        nc.vector.tensor_copy(identr[:], ident[:])

        # ---- Phase 0: h = x @ W^T ----
        w_sb = const_pool.tile([128, C], F32)
        nc.scalar.dma_start(out=w_sb[:], in_=w_d[:, :])
        psum_wt = psum_misc.tile([128, 128], F32, tag="pm")
        nc.tensor.transpose(psum_wt[:], w_sb[:], ident[:])
        wtr_sb = const_pool.tile([128, C], F32R)
        nc.vector.tensor_copy(wtr_sb[:], psum_wt[:])

        h_sb = h_pool.tile([128, NKT, C], F32R)
        XCH = 16  # x DMA chunks so h-compute overlaps the load
        NTX = NKT // XCH
        for xc in range(XCH):
            x_raw = xr_pool.tile([128, NTX, C], F32, tag="xraw")
            nc.scalar.dma_start(
                out=x_raw[:],
                in_=x_d[xc * NTX * 128 : (xc + 1) * NTX * 128, :].rearrange(
                    "(t p) c -> p t c", p=128
                ),
            )
            x_rnd = xr_pool.tile([128, NTX, C], F32R, tag="xrnd")
            nc.vector.tensor_copy(x_rnd[:], x_raw[:])  # fp32r rounding pass
            for tt in range(NTX):
                t = xc * NTX + tt
                psum_xt = psum_misc.tile([128, 128], F32R, tag="pm")
                nc.tensor.transpose(psum_xt[:], x_rnd[:, tt, :], identr[:])
                xt_sb = xt_pool.tile([128, 128], F32R)
                nc.vector.tensor_copy(xt_sb[:], psum_xt[:])
                psum_h = psum_misc.tile([128, 128], F32, tag="pm")
                nc.tensor.matmul(psum_h[:], xt_sb[:], wtr_sb[:], start=True, stop=True)
                nc.vector.tensor_copy(h_sb[:, t, :], psum_h[:])

        # h rows owned by this core (for the +I self-loop), exact fp32 path
        xo_sb = xr_pool.tile([128, R // 128, C], F32, tag="xo")
        nc.scalar.dma_start(
            out=xo_sb[:], in_=xo_d.rearrange("(t p) c -> p t c", p=128)
        )
        ho_sb = h_pool.tile([128, R // 128, C], F32)
        for t in range(R // 128):
            psum_xt = psum_misc.tile([128, 128], F32, tag="pm")
            nc.tensor.transpose(psum_xt[:], xo_sb[:, t, :], ident[:])
            xt_f = xt_pool.tile([128, 128], F32, tag="xtf")
            nc.vector.tensor_copy(xt_f[:], psum_xt[:])
            psum_h = psum_misc.tile([128, 128], F32, tag="pm")
            nc.tensor.matmul(
                psum_h[:], xt_f[:], wtr_sb[:].bitcast(F32), start=True, stop=True
            )
            nc.vector.tensor_copy(ho_sb[:, t, :], psum_h[:])

        # ---- Phase 1: main loop over (m-block, k-chunk) ----
        for blk in range(NBLK):
            pacc = psum_acc.tile([128, M_BLK], F32)
            for q in range(NQ):
                # load/multiply in half-chunks of 2 slabs: finer buffer
                # release keeps the DMA queue streaming without stalls
                halves = []
                for hb in range(2):
                    r0 = blk * M_BLK + hb * 256
                    adj_t = adj_pool.tile([128, 2, KQ], F32)
                    nc.sync.dma_start(
                        out=adj_t[:],
                        in_=adj_d[r0 : r0 + 256, q * KQ : (q + 1) * KQ].rearrange(
                            "(s p) k -> p s k", p=128
                        ),
                    )
                    mask_t = mask_pool.tile([128, 2, KQ], F32)
                    nc.sync.dma_start(
                        out=mask_t[:],
                        in_=mask_d[r0 : r0 + 256, q * KQ : (q + 1) * KQ].rearrange(
                            "(s p) k -> p s k", p=128
                        ),
                    )
                    prod_t = prod_pool.tile([128, 2, KQ], F32R)
                    nc.vector.tensor_mul(prod_t[:], adj_t[:], mask_t[:])
                    halves.append(prod_t)

                for kt in range(KT):
                    kg = q * KT + kt  # global k-tile index 0..63
                    psum_at = psum_tr.tile([128, M_BLK], F32R)
                    for s in range(S):
                        nc.tensor.transpose(
                            psum_at[:, s * 128 : (s + 1) * 128],
                            halves[s // 2][:, s % 2, kt * 128 : (kt + 1) * 128],
                            identr[:],
                        )
                    at_sb = at_pool.tile([128, M_BLK], F32R)
                    nc.scalar.copy(at_sb[:], psum_at[:])
                    nc.tensor.matmul(
                        pacc[:],
                        h_sb[:, kg, :],
                        at_sb[:],
                        start=(kg == 0),
                        stop=(kg == NKT - 1),
                    )

            # ---- finalize m-block: back-transpose out^T, add self-loop h ----
            outT_sb = fin_pool.tile([128, M_BLK], F32)
            nc.vector.tensor_copy(outT_sb[:], pacc[:])
            psum_nat = psum_fin.tile([128, M_BLK], F32)
            for s in range(S):
                nc.tensor.transpose(
                    psum_nat[:, s * 128 : (s + 1) * 128],
                    outT_sb[:, s * 128 : (s + 1) * 128],
                    ident[:],
                )
            out_sb = fin_pool.tile([128, S, C], F32)
            nc.vector.tensor_add(
                out_sb[:],
                psum_nat[:].rearrange("p (s c) -> p s c", s=S),
                ho_sb[:, blk * S : (blk + 1) * S, :],
            )
            nc.scalar.dma_start(
                out=out_d[blk * M_BLK : (blk + 1) * M_BLK, :].rearrange(
                    "(s p) c -> p s c", p=128
                ),
                in_=out_sb[:],
            )

    nc.compile()
    return nc


_NC_CACHE = None


def _get_nc():
    global _NC_CACHE
    if _NC_CACHE is None:
        _NC_CACHE = build_program()
    return _NC_CACHE


def kernel(x, adj, mask, W):
    x = np.ascontiguousarray(x, dtype=np.float32)
    adj = np.ascontiguousarray(adj, dtype=np.float32)
    mask = np.ascontiguousarray(mask, dtype=np.float32)
    W = np.ascontiguousarray(W, dtype=np.float32)

    nc = _get_nc()
    in_maps = []
    for i in range(NCORES):
        r0 = i * R
        in_maps.append(
            {
                "adj": adj[r0 : r0 + R],
                "mask": mask[r0 : r0 + R],
                "x": x,
                "x_own": x[r0 : r0 + R],
                "w": W,
            }
        )
    res = run_bass_kernel_spmd(nc, in_maps, list(range(NCORES)))
    return np.concatenate([res.results[i]["out"] for i in range(NCORES)], axis=0)
